# revision 63
# baseline (speedup 1.0000x reference)
"""Trainium2 Bass kernel for nn_BaselineModel_74509092651544 (CLRS-style MPNN).

Strategy
--------
Data-parallel over graphs: 32 graphs -> 8 cores x 4 graphs.  The dense
[B,N,N,H] message tensor is never materialized: only the ~61k unique
(graph,src,dst) edge slots survive the masked max, so the message MLP runs
on a padded CSR slot layout.

This version vs the earlier 114us baseline (~96us):
  * fp8 DoubleRow matmuls (0.5 cyc/row) for the gather stage: m1/m2
    (quantized fp8e4m3) ride as a 2-k-tile stationary pair against a
    column-interleaved Gsrc/Gdst one-hot moving tensor; the bond term
    uses a 24+24 row split of the one-hot counts.  1 cycle/slot on the
    PE instead of 3.
  * bf16 weights/activations everywhere precision allows (validated
    ~7.8e-3 rel err vs the 2e-2 gate).
  * One GLOBAL software-pipelined stream of tile-pair chunks across all
    L layers; per-graph-pair h/LayerNorm/m12 blocks are injected between
    chunk steps so layer boundaries overlap the next layer's slot work.
  * relu1 on ACT over paired PSUM banks; relu2 split ACT/DVE (Bresenham);
    segmented max on DVE; LN-norm scalars on the otherwise idle GPSIMD
    (plain tensor_scalar only - walrus rejects stt/Pool).
  * DP-optimized receiver grouping; DMAs ordered/merged so graph-0
    gather matrices land ~4us in (the cost model serializes all DMA).
  * Last layer pools straight out of the LN transpose PSUM (no hidden
    write-back); mean/ln_s folded into one scale column.
"""

import sys
import numpy as np

sys.path.insert(0, "/opt/trn_rl_repo")

B, N, H, L, E, OUT = 32, 128, 128, 3, 65536, 128
M = 8                 # NeuronCores
BL = B // M           # graphs per core
NEG = -1e9
EPS = 1e-5
AV, BV = 128, 16
ACT_RELU2_SHARE = 0.60   # fraction of relu2 ops on ACT (rest on DVE)

_CACHE = {}


# --------------------------------------------------------------------------
# Host preprocessing: pure integer / relayout work.
# --------------------------------------------------------------------------

def _ffd_pack(groups):
    sizes = [R * K for (_, R, K) in groups]
    order = np.argsort(-np.asarray(sizes), kind="stable")
    bins, place = [], [None] * len(groups)
    for gi in order:
        sz = sizes[gi]
        for t in range(len(bins)):
            if bins[t] + sz <= 512:
                place[gi] = (t, bins[t])
                bins[t] += sz
                break
        else:
            place[gi] = (len(bins), 0)
            bins.append(sz)
    return bins, place


def _dp_groups(Kp, c_slot, c_group, max_r=128):
    n = len(Kp)
    INF = float("inf")
    dp = [INF] * (n + 1)
    dp[n] = 0.0
    choice = [1] * (n + 1)
    for p in range(n - 1, -1, -1):
        K = int(Kp[p])
        mr = min(n - p, 512 // K, max_r)
        best, best_r = INF, 1
        for R in range(1, mr + 1):
            c = R * K * c_slot + c_group + dp[p + R]
            if c < best:
                best, best_r = c, R
        dp[p], choice[p] = best, best_r
    groups = []
    p = 0
    while p < n:
        R = choice[p]
        groups.append((p, R, int(Kp[p])))
        p += R
    return groups


def _fixed_groups(Kp, r0):
    groups, p, n = [], 0, len(Kp)
    while p < n:
        K = int(Kp[p])
        R = min(r0, n - p)
        while R * K > 512:
            R //= 2
        groups.append((p, R, K))
        p += R
    return groups


def _choose_groups(Kp):
    """Pick the candidate minimizing a per-graph-layer time proxy (ns)."""
    cands = [_fixed_groups(Kp, 16),
             _dp_groups(Kp, 4.0, 100.0),
             _dp_groups(Kp, 4.0, 170.0, 16),
             _dp_groups(Kp, 2.5, 170.0)]
    best, best_c = None, float("inf")
    for gs in cands:
        bins, _ = _ffd_pack(gs)
        S = sum(R * K for (_, R, K) in gs)
        c = S * 4.04 + len(gs) * 170.0 + len(bins) * 395.0
        if c < best_c:
            best, best_c = gs, c
    return best


def _prep(inputs):
    x = np.asarray(inputs["x"]).astype(np.int64)            # [B*N, 9]
    ea = np.asarray(inputs["edge_attr"]).astype(np.int64)   # [E, 3]
    ei = np.asarray(inputs["edge_index"]).astype(np.int64)  # [2, E]

    g = ei[0] // N
    s = ei[0] % N
    d = ei[1] % N
    key = (g * N + s) * N + d
    uniq, inv = np.unique(key, return_inverse=True)
    US = uniq.size
    ug = uniq // (N * N)
    us = (uniq // N) % N
    ud = uniq % N

    # bond one-hot counts per unique slot  [US, 48]
    oh48 = np.zeros((US, 48), np.float32)
    for c in range(3):
        np.add.at(oh48, (inv, ea[:, c] + 16 * c), 1.0)

    # unique in-degree per (graph, receiver)
    deg = np.zeros((B, N), np.int64)
    np.add.at(deg, (ug, ud), 1)

    # receiver relabeling: position p holds the p-th highest-degree receiver
    rho = np.argsort(-deg, axis=1, kind="stable")        # [B, N] pos -> orig
    rho_inv = np.argsort(rho, axis=1)                    # orig -> pos
    degS = -np.sort(-deg, axis=1)                        # [B, N] desc
    Kp = np.maximum(degS.max(axis=0), 1)                 # [N] non-increasing

    groups = _choose_groups(Kp)                          # (p0, R, K)
    tiles_used, place = _ffd_pack(groups)
    n_tiles = len(tiles_used)
    tile_w = [int(w) for w in tiles_used]
    tile_base = np.concatenate([[0], np.cumsum(tile_w)[:-1]]).astype(np.int64)
    S_graph = int(((sum(tile_w) + 15) // 16) * 16)
    S_core = BL * S_graph

    # per-position lookup tables
    col_base_of_pos = np.zeros(N, np.int64)   # first column of the receiver
    K_of_pos = np.zeros(N, np.int64)
    for gi, (p0, R, K) in enumerate(groups):
        t, off = place[gi]
        for r in range(R):
            col_base_of_pos[p0 + r] = tile_base[t] + off + r * K
            K_of_pos[p0 + r] = K

    # slots ordered by (g, d, s): contiguous per receiver
    order = np.lexsort((us, ud, ug))
    og, od, osl = ug[order], ud[order], order
    osrc = us[order]
    recv_id = og * N + od
    first = np.concatenate([[0], np.flatnonzero(np.diff(recv_id)) + 1])
    k_rank = np.arange(len(og)) - first[np.searchsorted(recv_id[first], recv_id)]

    pos = rho_inv[og, od]
    core_r = og // BL
    col_r = (og % BL) * S_graph + col_base_of_pos[pos] + k_rank

    # padding: receivers with deg < K duplicate their first slot
    fg, fd = og[first], od[first]
    fpos = rho_inv[fg, fd]
    fdeg = deg[fg, fd]
    fK = K_of_pos[fpos]
    padc = (fK - fdeg).astype(np.int64)
    assert (padc >= 0).all()
    rep = np.repeat(np.arange(len(first)), padc)
    kpad = np.arange(len(rep)) - np.repeat(
        np.concatenate([[0], np.cumsum(padc)[:-1]]), padc
    ) + np.repeat(fdeg, padc)
    pg = fg[rep]
    core_p = pg // BL
    col_p = (pg % BL) * S_graph + col_base_of_pos[fpos[rep]] + kpad
    slot_p = osl[first][rep]
    src_p = osrc[first][rep]

    a_core = np.concatenate([core_r, core_p])
    a_col = np.concatenate([col_r, col_p])
    a_slot = np.concatenate([osl, slot_p])
    a_srcnew = np.concatenate([rho_inv[og, osrc], rho_inv[pg, src_p]])
    a_dstpos = np.concatenate([pos, fpos[rep]])

    import ml_dtypes
    FP8 = ml_dtypes.float8_e4m3fn
    flat = a_core * S_core + a_col
    # column-interleaved src/dst one-hot k-tiles: [M, 128, S_core*2]
    # (column s occupies bytes [2s, 2s+1]: k=0 -> Gsrc, k=1 -> Gdst)
    Gpair = np.zeros((M * S_core, 2, 128), np.float32)
    Gpair[flat, 0, a_srcnew] = 1.0
    Gpair[flat, 1, a_dstpos] = 1.0
    Gpair = np.ascontiguousarray(
        Gpair.reshape(M, S_core, 2, 128).transpose(0, 3, 1, 2)
        .reshape(M, 128, 2 * S_core)).astype(FP8)
    # bond count k-tiles (rows 0:24 / 24:48), column-interleaved:
    # [M, 24, S_core*2]
    SOH = np.zeros((M * S_core, 48), np.float32)
    SOH[flat] = oh48[a_slot]
    assert float(SOH.max()) <= 16.0
    SOH = np.ascontiguousarray(
        SOH.reshape(M, S_core, 2, 24).transpose(0, 3, 1, 2)
        .reshape(M, 24, 2 * S_core)).astype(FP8)

    # atom one-hot per core: [M, 9, AV, BL*N] in relabeled node order
    gg = np.repeat(np.arange(B), N)
    pp = np.tile(np.arange(N), B)
    orig = gg * N + rho[gg, pp]                    # [B*N] column -> orig node
    XOH = np.zeros((M, 9, AV, BL * N), np.float32)
    mcol = np.tile(np.arange(BL * N), M)
    mcore = np.repeat(np.arange(M), BL * N)
    for c in range(9):
        XOH[mcore, c, x[orig, c], mcol] = 1.0
    # one flat [AV, 9*BL*N] tensor per core -> single DMA
    XOH = np.ascontiguousarray(
        XOH.transpose(0, 2, 1, 3).reshape(M, AV, 9 * BL * N)).astype(FP8)

    # empty receivers (deg==0) -> need NEG mask path
    empt = (deg == 0)
    has_empty = bool(empt.any())
    maskrow = np.ones((M, BL * N), np.float32)
    negrow = np.zeros((M, BL * N), np.float32)
    if has_empty:
        eg, en = np.nonzero(empt)
        epos = rho_inv[eg, en]
        maskrow[eg // BL, (eg % BL) * N + epos] = 0.0
        negrow[eg // BL, (eg % BL) * N + epos] = NEG

    struct = dict(
        S_graph=S_graph, S_core=S_core, n_tiles=n_tiles,
        groups=[(p0, R, K, place[gi][0], place[gi][1])
                for gi, (p0, R, K) in enumerate(groups)],
        tile_w=tuple(tile_w), tile_base=tuple(int(b) for b in tile_base),
        has_empty=has_empty,
    )
    percore = dict(Gpair=Gpair, SOH=SOH, XOH=XOH,
                   maskrow=maskrow, negrow=negrow)
    return struct, percore


def _weight_arrays(inputs):
    import ml_dtypes
    BF16 = ml_dtypes.bfloat16
    f32 = np.float32

    def blob(cols):
        wmap = {}
        off = 0
        for name, arr in cols:
            wmap[name] = (off, arr.shape[1])
            off += arr.shape[1]
        data = np.concatenate([a for _, a in cols], 1)
        return np.ascontiguousarray(data), wmap

    Wm1 = np.asarray(inputs["Wm1"], f32)
    Wm2 = np.asarray(inputs["Wm2"], f32)
    bond_T = np.asarray(inputs["bond_emb"], f32).reshape(48, H).T  # [128, 48]
    # chunk 1: everything layer-0's slot phase needs (DMA'd first)
    cols_b = [("bondT", bond_T),
              ("We_0", np.asarray(inputs["We"], f32)[0]),
              ("m12_0_0",
               np.concatenate([Wm1[0, 0:128], Wm2[0, 0:128]], 1)),
              ("Wp1_0", np.asarray(inputs["Wp1"], f32)[0]),
              ("Wp2_0", np.asarray(inputs["Wp2"], f32)[0])]
    wb_split = sum(a.shape[1] for _, a in cols_b)
    # chunk 2: the rest (lands before layer-0's h phase)
    cols_b.append(("Wo1_0_0", np.asarray(inputs["Wo1"], f32)[0, 0:128]))
    for l in range(1, L):
        cols_b.append((f"We_{l}", np.asarray(inputs["We"], f32)[l]))
        cols_b.append((f"m12_{l}_0",
                       np.concatenate([Wm1[l, 0:128], Wm2[l, 0:128]], 1)))
        cols_b.append((f"m12_{l}_1",
                       np.concatenate([Wm1[l, 128:256], Wm2[l, 128:256]], 1)))
        cols_b.append((f"Wp1_{l}", np.asarray(inputs["Wp1"], f32)[l]))
        cols_b.append((f"Wp2_{l}", np.asarray(inputs["Wp2"], f32)[l]))
        cols_b.append((f"Wo1_{l}_0", np.asarray(inputs["Wo1"], f32)[l, 0:128]))
        cols_b.append((f"Wo1_{l}_1", np.asarray(inputs["Wo1"], f32)[l, 128:256]))
    for l in range(L):
        cols_b.append((f"Wo2_{l}", np.asarray(inputs["Wo2"], f32)[l]))
    cols_b.append(("idn1", np.concatenate(
        [np.eye(128, dtype=f32), np.ones((128, 1), f32)], 1)))
    wb, wbmap = blob(cols_b)

    cols_r = [("Wh1", np.asarray(inputs["Wh1"], f32)),
              ("Wh2", np.asarray(inputs["Wh2"], f32)),
              ("idn", np.eye(128, dtype=f32))]
    wr, wrmap = blob(cols_r)

    A = {}
    A["wb"] = wb.astype(BF16)
    A["wr"] = wr
    A["_wbmap"] = wbmap
    A["_wrmap"] = wrmap
    A["_wbsplit"] = wb_split

    at = np.asarray(inputs["atom_emb"], f32).transpose(1, 0, 2).reshape(AV, 9 * H)
    A["atomb"] = np.ascontiguousarray(at).astype(BF16)

    # bias columns [128, 33]: 4 pre-terms x L, 2 o-terms x L, bh1, bh2, eps,
    # ln_s x L, ln_b x L, bp1 x L, bp2 x L (same layout as baseline)
    bc = np.zeros((H, 34), f32)
    bc[:, 26] = EPS
    bc[:, 33] = np.asarray(inputs["ln_s"], f32)[L - 1] / N
    bc[:, 27:30] = np.asarray(inputs["ln_s"], f32).T
    bc[:, 30:33] = np.asarray(inputs["ln_b"], f32).T
    for l in range(L):
        bc[:, 4 * l + 0] = np.asarray(inputs["bm1"], f32)[l]
        bc[:, 4 * l + 1] = np.asarray(inputs["bm2"], f32)[l]
        bc[:, 4 * l + 2] = np.asarray(inputs["be"], f32)[l]
        bc[:, 4 * l + 3] = np.asarray(inputs["bg"], f32)[l]
        bc[:, 12 + 2 * l + 0] = np.asarray(inputs["bo1"], f32)[l]
        bc[:, 12 + 2 * l + 1] = np.asarray(inputs["bo2"], f32)[l]
        bc[:, 18 + l] = np.asarray(inputs["bp1"], f32)[l]
        bc[:, 23 + l] = np.asarray(inputs["bp2"], f32)[l]
    bc[:, 21] = np.asarray(inputs["bh1"], f32)
    bc[:, 22] = np.asarray(inputs["bh2"], f32)[:H]
    A["bias_cols"] = bc
    A["bh2_full"] = np.ascontiguousarray(
        np.asarray(inputs["bh2"], f32).reshape(OUT, 1))
    bp2f = np.zeros((H, 4), f32)
    bp2f[:, :L] = np.asarray(inputs["bp2"], f32).T
    A["bp2f"] = bp2f.astype(BF16)
    return A


# --------------------------------------------------------------------------
# Bass program.
# --------------------------------------------------------------------------

def _build_program(struct, wbmap, wrmap, wbc, wrc, wbsplit):
    import concourse.bacc as bacc
    import concourse.mybir as mybir
    import concourse.tile as tile

    F32 = mybir.dt.float32
    nc = bacc.Bacc("TRN2", target_bir_lowering=False, debug=False)

    BF16 = mybir.dt.bfloat16
    FP8 = mybir.dt.float8e4
    F32R = mybir.dt.float32r
    S_core = struct["S_core"]
    d = {}
    d["d_gpair"] = nc.dram_tensor("gpair", [128, 2 * S_core], FP8,
                                  kind="ExternalInput")
    d["d_soh"] = nc.dram_tensor("soh", [24, 2 * S_core], FP8,
                                kind="ExternalInput")
    d["d_xoh"] = nc.dram_tensor("xoh", [AV, 9 * BL * N], FP8,
                                kind="ExternalInput")
    d["d_atomb"] = nc.dram_tensor("atomb", [AV, 9 * H], BF16,
                                  kind="ExternalInput")
    d["d_wb"] = nc.dram_tensor("wb", [128, wbc], BF16, kind="ExternalInput")
    d["d_wr"] = nc.dram_tensor("wr", [128, wrc], F32R, kind="ExternalInput")
    d["d_bc"] = nc.dram_tensor("bias_cols", [H, 34], F32, kind="ExternalInput")
    d["d_bh2"] = nc.dram_tensor("bh2_full", [OUT, 1], F32, kind="ExternalInput")
    d["d_bp2f"] = nc.dram_tensor("bp2f", [H, 4], BF16, kind="ExternalInput")
    d["d_mask"] = nc.dram_tensor("maskrow", [1, BL * N], F32,
                                 kind="ExternalInput")
    d["d_neg"] = nc.dram_tensor("negrow", [1, BL * N], F32,
                                kind="ExternalInput")
    d["d_out"] = nc.dram_tensor("out", [OUT, BL], F32, kind="ExternalOutput")

    with tile.TileContext(nc) as tc:
        _emit(tc, nc, d, struct, wbmap, wrmap, mybir, wbsplit)
    nc.compile()
    return nc


def _emit(tc, nc, d, struct, wbmap, wrmap, mybir, wbsplit):
    import contextlib
    from collections import defaultdict
    ctx = contextlib.ExitStack()
    F32 = mybir.dt.float32
    F32R = mybir.dt.float32r
    BF16 = mybir.dt.bfloat16
    FP8 = mybir.dt.float8e4
    AF = mybir.ActivationFunctionType
    ALU = mybir.AluOpType
    AX = mybir.AxisListType
    DR = mybir.MatmulPerfMode.DoubleRow

    S_graph = struct["S_graph"]
    S_core = struct["S_core"]
    n_tiles = struct["n_tiles"]
    groups = struct["groups"]
    tile_w = struct["tile_w"]
    tile_base = struct["tile_base"]
    has_empty = struct["has_empty"]

    pG = ctx.enter_context(tc.tile_pool(name="pG", bufs=1))
    pW = ctx.enter_context(tc.tile_pool(name="pW", bufs=1))
    pAct = ctx.enter_context(tc.tile_pool(name="pAct", bufs=3))
    pNM = ctx.enter_context(tc.tile_pool(name="pNM", bufs=1))
    pMB = ctx.enter_context(tc.tile_pool(name="pMB", bufs=2))
    pLN = ctx.enter_context(tc.tile_pool(name="pLN", bufs=1))
    pIn = ctx.enter_context(tc.tile_pool(name="pIn", bufs=2))
    ps_a = ctx.enter_context(tc.tile_pool(name="ps_a", bufs=2, space="PSUM"))
    ps_b = ctx.enter_context(tc.tile_pool(name="ps_b", bufs=2, space="PSUM"))
    ps_c = ctx.enter_context(tc.tile_pool(name="ps_c", bufs=2, space="PSUM"))

    def psA(dt=F32):
        """Double-bank tile: pre pairs (also m12/h_ps/nf_ps, half-used)."""
        return ps_a.tile([128, 1024], dt, name="psA", tag="psA")

    def psB(dt=F32):
        return ps_b.tile([128, 512], dt, name="psB", tag="psB")

    def psC(dt=F32):
        return ps_c.tile([128, 512], dt, name="psC", tag="psC")

    # ---- resident tiles
    gpair_sb = pG.tile([128, 2 * S_core], FP8, name="gpair_sb")
    soh_sb = pG.tile([24, 2 * S_core], FP8, name="soh_sb")

    def k2(tile_sb, c0, w):
        """[p, 2*(c0..c0+w)] interleaved slice -> [p, k=2, s=w] AP."""
        return tile_sb[:, 2 * c0:2 * (c0 + w)].rearrange(
            "p (s k) -> p k s", k=2)

    wbc = sum(w for _, w in wbmap.values())
    wb_sb = pW.tile([128, wbc], BF16, name="wb_sb")
    wr_sb = pW.tile([128, sum(w for _, w in wrmap.values())], F32R,
                    name="wr_sb")
    bc_sb = pW.tile([H, 34], F32, name="bc_sb")
    bh2_sb = pW.tile([OUT, 1], F32, name="bh2_sb")
    bp2f_sb = pW.tile([H, 4], BF16, name="bp2f_sb")
    atomb_sb = pW.tile([AV, 9 * H], BF16, name="atomb_sb")

    def WB(name):
        off, w = wbmap[name]
        return wb_sb[:, off:off + w]

    def WR(name):
        off, w = wrmap[name]
        return wr_sb[:, off:off + w]

    # ---- DMAs, warmup-critical first
    nc.sync.dma_start(wb_sb[:, 0:wbsplit], d["d_wb"].ap()[:, 0:wbsplit])
    nc.sync.dma_start(bc_sb[:], d["d_bc"].ap())
    nc.sync.dma_start(atomb_sb[:], d["d_atomb"].ap())
    xoh_all = pIn.tile([AV, 9 * BL * N], FP8, name="xoh_all", tag="xoh",
                       bufs=1)
    c3 = 3 * BL * N
    nc.sync.dma_start(xoh_all[:, 0:c3], d["d_xoh"].ap()[:, 0:c3])
    nc.sync.dma_start(xoh_all[:, c3:], d["d_xoh"].ap()[:, c3:])
    sl = slice(0, 2 * S_graph)
    nc.sync.dma_start(gpair_sb[:, sl], d["d_gpair"].ap()[:, sl])
    nc.sync.dma_start(soh_sb[:, sl], d["d_soh"].ap()[:, sl])
    slr = slice(2 * S_graph, 2 * BL * S_graph)
    nc.sync.dma_start(gpair_sb[:, slr], d["d_gpair"].ap()[:, slr])
    nc.sync.dma_start(soh_sb[:, slr], d["d_soh"].ap()[:, slr])
    nc.sync.dma_start(wb_sb[:, wbsplit:], d["d_wb"].ap()[:, wbsplit:])
    nc.sync.dma_start(wr_sb[:], d["d_wr"].ap())
    nc.sync.dma_start(bp2f_sb[:], d["d_bp2f"].ap())
    nc.sync.dma_start(bh2_sb[:], d["d_bh2"].ap())
    if has_empty:
        mrow_sb = pW.tile([1, BL * N], F32, name="mrow_sb")
        nc.sync.dma_start(mrow_sb[:], d["d_mask"].ap())
        nrow_sb = pW.tile([1, BL * N], F32, name="nrow_sb")
        nc.sync.dma_start(nrow_sb[:], d["d_neg"].ap())
        mask_bc = pW.tile([128, BL * N], F32, name="mask_bc")
        nc.gpsimd.partition_broadcast(mask_bc[:], mrow_sb[:])
        neg_bc = pW.tile([128, BL * N], F32, name="neg_bc")
        nc.gpsimd.partition_broadcast(neg_bc[:], nrow_sb[:])

    # pin the activation table to the set that has Relu+Copy+Identity+Sqrt
    sq_dummy = pW.tile([128, 1], F32, name="sq_dummy")
    nc.scalar.activation(sq_dummy[:], bc_sb[:, 26:27], AF.Sqrt)

    # bias prework (bc only)
    bias_pre = pW.tile([128, L], F32, name="bias_pre")
    nc.vector.tensor_reduce(
        bias_pre[:], bc_sb[:, 0:4 * L].rearrange("p (l f) -> p l f", l=L),
        axis=AX.X, op=ALU.add)
    bo12 = pW.tile([128, L], F32, name="bo12")
    nc.vector.tensor_reduce(
        bo12[:], bc_sb[:, 12:12 + 2 * L].rearrange("p (l f) -> p l f", l=L),
        axis=AX.X, op=ALU.add)

    # ---- node features (feature-major), bf16
    nf_ps = psA()
    for c in range(9):
        nc.tensor.matmul(nf_ps[:, 0:512], atomb_sb[:, c * H:(c + 1) * H],
                         xoh_all[:, c * BL * N:(c + 1) * BL * N],
                         start=(c == 0), stop=(c == 8))
    nf = pNM.tile([128, BL * N], BF16, name="nf")
    nc.scalar.activation(nf[:], nf_ps[:, 0:512], AF.Copy)

    # layer-0 m12 (z = [nf, 0]) + fp8 copies, and layer-0 bond weights
    def emit_bw(l):
        bw_ps = psB()
        nc.tensor.matmul(bw_ps[0:24, 0:128], WB("bondT")[:, 0:24],
                         WB(f"We_{l}"), start=True, stop=True)
        nc.tensor.matmul(bw_ps[0:24, 128:256], WB("bondT")[:, 24:48],
                         WB(f"We_{l}"), start=True, stop=True,
                         skip_group_check=True)
        bw24 = pMB.tile([24, 256], FP8, name="bw24", tag=f"bw24_{l}", bufs=1)
        nc.scalar.activation(bw24[:], bw_ps[0:24, 0:256], AF.Copy)
        return bw24

    bw24_l = {0: emit_bw(0)}

    mt_l = {0: pMB.tile([128, BL * 256], FP8, name="mt0", tag="mt", bufs=2)}
    for half in range(2):
        ps = psA()
        for gg in (2 * half, 2 * half + 1):
            off = (gg % 2) * 256
            gsl = slice(gg * N, (gg + 1) * N)
            nc.tensor.matmul(ps[:, off:off + 256], nf[:, gsl],
                             WB("m12_0_0"), start=True, stop=True)
        nc.scalar.activation(mt_l[0][:, half * 512:(half + 1) * 512],
                             ps[:, 0:512], AF.Copy)

    # relu2 engine split (Bresenham across all layers)
    state = dict(relu2_acc=0.0)

    def emit_chunk_pre(l, gg, tt):
        pre = psA()
        mt_g = mt_l[l][:, gg * 256:(gg + 1) * 256].rearrange(
            "p (k f) -> p k f", k=2)
        bw_k = bw24_l[l][:].rearrange("p (k f) -> p k f", k=2)
        for i, t in enumerate(tt):
            w = tile_w[t]
            if i + 1 < len(tt):
                # widen to fill the 512-col half: the paired relu below must
                # not read unwritten PSUM (extra gather cols are in-bounds;
                # their relu output lands in unread slots of msgs1)
                assert tile_base[t] + 512 <= S_graph
                w = 512
            c0 = gg * S_graph + tile_base[t]
            sl = slice(i * 512, i * 512 + w)
            nc.tensor.matmul(pre[:, sl], mt_g, k2(gpair_sb, c0, w),
                             start=True, stop=False, perf_mode=DR)
            nc.tensor.matmul(pre[:, sl], bw_k, k2(soh_sb, c0, w),
                             start=False, stop=True, perf_mode=DR)
        wtot = 512 * (len(tt) - 1) + tile_w[tt[-1]]
        msgs1 = pAct.tile([128, 1024], BF16, name="msgs1", tag="msgs1",
                          bufs=2)
        nc.scalar.activation(msgs1[:, 0:wtot], pre[:, 0:wtot], AF.Relu,
                             bias=bias_pre[:, l:l + 1])
        return msgs1

    def emit_chunk_p1(l, gg, tt, msgs1):
        outs = []
        for i, t in enumerate(tt):
            w = tile_w[t]
            p1 = psB()
            nc.tensor.matmul(p1[:, 0:w], WB(f"Wp1_{l}"),
                             msgs1[:, i * 512:i * 512 + w],
                             start=True, stop=True)
            msgs2 = pAct.tile([128, 512], BF16, name="msgs2", tag="msgs2",
                              bufs=3)
            state["relu2_acc"] += ACT_RELU2_SHARE
            if state["relu2_acc"] >= 1.0:
                state["relu2_acc"] -= 1.0
                nc.scalar.activation(msgs2[:, 0:w], p1[:, 0:w], AF.Relu,
                                     bias=bc_sb[:, 18 + l:19 + l])
            else:
                nc.vector.tensor_scalar(msgs2[:, 0:w], p1[:, 0:w],
                                        bc_sb[:, 18 + l:19 + l], 0.0,
                                        op0=ALU.add, op1=ALU.max)
            outs.append(msgs2)
        return outs

    def emit_chunk_p2(l, gg, tt, msgs2s, msgs_max):
        for i, t in enumerate(tt):
            w = tile_w[t]
            p2 = psC()
            nc.tensor.matmul(p2[:, 0:w], WB(f"Wp2_{l}"), msgs2s[i][:, 0:w],
                             start=True, stop=True)
            for (p0, R, K, gt, off) in groups:
                if gt != t:
                    continue
                nc.vector.tensor_reduce(
                    msgs_max[:, gg * N + p0: gg * N + p0 + R],
                    p2[:, off:off + R * K].rearrange("p (r k) -> p r k", r=R),
                    axis=AX.X, op=ALU.max)

    bias_h_l = {}
    ge_sum = pLN.tile([128, BL], F32, name="ge_sum", tag="ge_sum")

    hid = {0: None}
    TL = {}
    for l in range(L):
        TL[l] = dict(
            msgs_max=pLN.tile([128, BL * N], BF16, name="msgs_max",
                              tag="msgs_max", bufs=2),
            hn=pLN.tile([128, BL * (N + 1)], F32, name="hn", tag="hn",
                        bufs=1),
            hsq=pLN.tile([128, BL * N], BF16, name="hsq", tag="hsq", bufs=1),
            sumsq=pLN.tile([128, BL], F32, name="sumsq", tag="sumsq"),
            negmean=pLN.tile([128, BL], F32, name="negmean", tag="negmean"),
            msq=pLN.tile([128, BL], F32, name="msq", tag="msq"),
            var=pLN.tile([128, BL], F32, name="var", tag="var"),
            std=pLN.tile([128, BL], F32, name="std", tag="std"),
            rstd=pLN.tile([128, BL], F32, name="rstd", tag="rstd"),
            h_fm=pLN.tile([128, BL * N], BF16, name="h_fm", tag="h_fm",
                          bufs=1),
            hid_new=(pNM.tile([128, BL * N], BF16, name=f"hid{l + 1}",
                              tag=f"hid{(l + 1) % 2}")
                     if l + 1 < L else None),
        )
        hid[l + 1] = TL[l]["hid_new"]

    def mk_block_h(l, pair):
        def fn():
            t = TL[l]
            msgs_max = t["msgs_max"]
            if l not in bias_h_l:
                if has_empty:
                    bias_h_l[l] = bo12[:, l:l + 1]
                else:
                    bh_ps = psC()
                    nc.tensor.matmul(bh_ps[:, 0:2], WB(f"Wo2_{l}"),
                                     bp2f_sb[:, l:l + 2],
                                     start=True, stop=True)
                    bh = pMB.tile([128, 1], F32, name="bias_h",
                                  tag=f"bias_h{l}", bufs=1)
                    nc.vector.tensor_tensor(bh[:], bh_ps[:, 0:1],
                                            bo12[:, l:l + 1], op=ALU.add)
                    bias_h_l[l] = bh[:]
            msgs_src = msgs_max
            if has_empty:
                mmf = pLN.tile([128, BL * N], F32, name="mmf", tag="mmf",
                               bufs=1)
                nc.vector.scalar_tensor_tensor(
                    mmf[:], msgs_max[:], bc_sb[:, 23 + l:24 + l],
                    mask_bc[:], op0=ALU.add, op1=ALU.mult)
                nc.vector.tensor_tensor(mmf[:], mmf[:], neg_bc[:],
                                        op=ALU.add)
                mmb = pLN.tile([128, BL * N], BF16, name="mmb", tag="mmb",
                               bufs=1)
                nc.scalar.activation(mmb[:], mmf[:], AF.Copy)
                msgs_src = mmb
            g0 = pair[0]
            psl = slice(g0 * N, (g0 + 2) * N)
            h_ps = psA()
            nc.tensor.matmul(h_ps[:, 0:256], WB(f"Wo1_{l}_0"),
                             nf[:, psl], start=True, stop=False)
            if l > 0:
                nc.tensor.matmul(h_ps[:, 0:256], WB(f"Wo1_{l}_1"),
                                 hid[l][:, psl], start=False, stop=False)
            nc.tensor.matmul(h_ps[:, 0:256], WB(f"Wo2_{l}"),
                             msgs_src[:, psl], start=False, stop=True)
            nc.scalar.activation(t["h_fm"][:, psl], h_ps[:, 0:256], AF.Relu,
                                 bias=bias_h_l[l])
        return fn

    def mk_block_tp(l, pair):
        def fn():
            t = TL[l]
            hn, hsq = t["hn"], t["hsq"]
            g0 = pair[0]
            ssl = slice(g0, g0 + 2)
            W1 = N + 1
            tp_ps = psB(F32)
            for i, gg in enumerate(pair):
                # regular matmul vs [I | 1]: cols 0:128 transpose h to
                # node-major, col 128 = per-node feature sum
                nc.tensor.matmul(tp_ps[:, i * W1:(i + 1) * W1],
                                 t["h_fm"][:, gg * N:(gg + 1) * N],
                                 WB("idn1"), start=True, stop=True)
            nc.vector.tensor_scalar(hn[:, g0 * W1:(g0 + 2) * W1],
                                    tp_ps[:, 0:2 * W1],
                                    0.0, None, op0=ALU.add)
            for i, gg in enumerate(pair):
                gsn = slice(gg * W1, gg * W1 + N)
                nc.vector.scalar_tensor_tensor(
                    hsq[:, gg * N:(gg + 1) * N], hn[:, gsn], 0.0, hn[:, gsn],
                    op0=ALU.add, op1=ALU.mult,
                    accum_out=t["sumsq"][:, gg:gg + 1])
                nc.gpsimd.tensor_scalar(
                    t["negmean"][:, gg:gg + 1],
                    hn[:, gg * W1 + N:gg * W1 + N + 1],
                    -1.0 / H, None, op0=ALU.mult)
            nc.vector.tensor_tensor(t["msq"][:, ssl], t["negmean"][:, ssl],
                                    t["negmean"][:, ssl], op=ALU.mult)
            nc.vector.scalar_tensor_tensor(
                t["var"][:, ssl], t["sumsq"][:, ssl], 1.0 / H,
                t["msq"][:, ssl], op0=ALU.mult, op1=ALU.subtract)
            nc.scalar.activation(t["std"][:, ssl], t["var"][:, ssl], AF.Sqrt,
                                 bias=bc_sb[:, 26:27])
            nc.vector.reciprocal(t["rstd"][:, ssl], t["std"][:, ssl])
        return fn

    def mk_block_norm(l, pair):
        def fn():
            t = TL[l]
            g0 = pair[0]
            psl = slice(g0 * N, (g0 + 2) * N)
            tp2_ps = psC(F32R)
            for i, gg in enumerate(pair):
                hnorm = pLN.tile([128, 128], F32R, name="hnorm",
                                 tag="hnorm", bufs=2)
                nc.gpsimd.tensor_scalar(hnorm[:],
                                        t["hn"][:, gg * (N + 1):
                                                gg * (N + 1) + N],
                                        t["negmean"][:, gg:gg + 1],
                                        t["rstd"][:, gg:gg + 1],
                                        op0=ALU.add, op1=ALU.mult)
                nc.tensor.transpose(tp2_ps[:, i * 128:(i + 1) * 128],
                                    hnorm[:], WR("idn"))
            if l + 1 < L:
                nc.vector.tensor_scalar(t["hid_new"][:, psl],
                                        tp2_ps[:, 0:256].bitcast(F32),
                                        bc_sb[:, 27 + l:28 + l],
                                        bc_sb[:, 30 + l:31 + l],
                                        op0=ALU.mult, op1=ALU.add)
            else:
                # pool directly: ge += mean_n(tp2) * ln_s + ln_b (per pair)
                nc.vector.tensor_reduce(
                    ge_sum[:, g0:g0 + 2],
                    tp2_ps[:, 0:256].bitcast(F32).rearrange(
                        "p (g n) -> p g n", g=2),
                    axis=AX.X, op=ALU.add)
        return fn

    def mk_block_next(l, pair):
        def fn():
            nl = l + 1
            if nl >= L:
                return
            if nl not in bw24_l:
                bw24_l[nl] = emit_bw(nl)
                mt_l[nl] = pMB.tile([128, BL * 256], FP8,
                                    name=f"mt{nl}", tag="mt", bufs=2)
            ps = psA()
            for i, gg in enumerate(pair):
                off = i * 256
                gsl = slice(gg * N, (gg + 1) * N)
                nc.tensor.matmul(ps[:, off:off + 256], nf[:, gsl],
                                 WB(f"m12_{nl}_0"), start=True, stop=False)
                nc.tensor.matmul(ps[:, off:off + 256], hid[nl][:, gsl],
                                 WB(f"m12_{nl}_1"), start=False, stop=True)
            g0 = pair[0]
            nc.scalar.activation(
                mt_l[nl][:, g0 * 256:(g0 + 2) * 256], ps[:, 0:512],
                AF.Copy)
        return fn

    # chunk construction: pair widest (widened to its full bank) with
    # narrowest (its true width bounds the relu read)
    order_t = sorted(range(n_tiles), key=lambda t: -tile_w[t])
    tile_chunks = []
    avail = list(order_t)
    while len(avail) >= 2:
        f = avail[0]
        if tile_base[f] + 512 <= S_graph:
            tile_chunks.append((f, avail[-1]))
            avail = avail[1:-1]
        else:
            tile_chunks.append((f,))
            avail = avail[1:]
    if avail:
        tile_chunks.append((avail[0],))
    cpg = len(tile_chunks)
    nUL = BL * cpg

    units = [(l, gg, tt) for l in range(L)
             for gg in range(BL) for tt in tile_chunks]
    nU = len(units)
    post = defaultdict(list)
    for l in range(L):
        for pair in ((0, 1), (2, 3)):
            s0 = l * nUL + (pair[1] + 1) * cpg - 1 + 2
            post[s0].append(mk_block_h(l, pair))
            post[s0 + 1].append(mk_block_tp(l, pair))
            post[s0 + 2].append(mk_block_norm(l, pair))
            post[s0 + 3].append(mk_block_next(l, pair))
    st1, st2 = {}, {}
    for step in range(nU + 7):
        if step < nU:
            l, gg, tt = units[step]
            st1[step] = emit_chunk_pre(l, gg, tt)
        if 0 <= step - 1 < nU:
            u = step - 1
            l, gg, tt = units[u]
            st2[u] = emit_chunk_p1(l, gg, tt, st1.pop(u))
        if 0 <= step - 2 < nU:
            u = step - 2
            l, gg, tt = units[u]
            emit_chunk_p2(l, gg, tt, st2.pop(u), TL[l]["msgs_max"])
        for fn in post.pop(step, []):
            fn()

    # ---- pooling + prediction MLP (ge_sum accumulated in block_norm)
    ge = pLN.tile([128, BL], F32R, name="ge", tag="ge")
    nc.vector.tensor_scalar(ge[:], ge_sum[:], bc_sb[:, 33:34],
                            bc_sb[:, 30 + L - 1:31 + L - 1],
                            op0=ALU.mult, op1=ALU.add)
    o1 = psA()
    nc.tensor.matmul(o1[:, 0:BL], WR("Wh1"), ge[:], start=True, stop=True)
    t1 = pLN.tile([128, BL], F32R, name="t1", tag="t1")
    nc.scalar.activation(t1[:], o1[:, 0:BL], AF.Relu, bias=bc_sb[:, 21:22])
    o2 = psB()
    nc.tensor.matmul(o2[:, 0:BL], WR("Wh2"), t1[:], start=True, stop=True)
    out_sb = pLN.tile([OUT, BL], F32, name="out_sb", tag="out_sb")
    nc.scalar.activation(out_sb[:], o2[:, 0:BL], AF.Identity, bias=bh2_sb[:])
    nc.sync.dma_start(d["d_out"].ap(), out_sb[:])
    ctx.close()


# --------------------------------------------------------------------------
# Entry point.
# --------------------------------------------------------------------------

def build(inputs):
    struct, percore = _prep(inputs)
    A = _weight_arrays(inputs)
    wbmap = A.pop("_wbmap")
    wrmap = A.pop("_wrmap")
    wbsplit = A.pop("_wbsplit")
    key = (struct["S_graph"], struct["n_tiles"],
           tuple(struct["groups"]), struct["tile_w"], struct["has_empty"])
    if key not in _CACHE:
        _CACHE[key] = _build_program(struct, wbmap, wrmap,
                                     A["wb"].shape[1], A["wr"].shape[1],
                                     wbsplit)
    nc = _CACHE[key]

    in_maps = []
    for c in range(M):
        im = dict(
            gpair=percore["Gpair"][c], soh=percore["SOH"][c],
            xoh=percore["XOH"][c],
            maskrow=percore["maskrow"][c:c + 1],
            negrow=percore["negrow"][c:c + 1],
        )
        for k, v in A.items():
            im[k] = v
        in_maps.append(im)
    return nc, in_maps, struct


def kernel(**inputs):
    from concourse import bass_utils
    nc, in_maps, struct = build(inputs)
    res = bass_utils.run_bass_kernel_spmd(nc, in_maps, core_ids=list(range(M)))
    out = np.zeros((B, OUT), np.float32)
    for c in range(M):
        out[c * BL:(c + 1) * BL] = res.results[c]["out"].T
    return out
